# revision 66
# baseline (speedup 1.0000x reference)
"""GNN message passing + global softmax attention + MLP head on 8 TRN2 NeuronCores.

v2 strategy (node/dst-sharded SPMD, single program, rank enters via data):
  - GraphConv aggregation as block-dense adjacency matmuls (fp8 count matrix,
    SBUF-resident across both layers), same as v1.
  - Transposed-score attention: S^T[k, q] = K'^T Q is computed with k on
    partitions so the exp'd tiles feed the PV matmul directly (lhsT = V tile,
    rhs = exp tile) -- no DMA transpose of the score tiles at all.
  - Numerical range without a per-row max pass: host fits a linear model
    c_q ~= Q_q . u + beta to the (subsampled) row max of the scores, then
    folds -sqrt(128)*u into the K bias (per-q shift is constant along k, so
    softmax is mathematically unchanged) and -beta into the exp bias.
  - Softmax denominator accumulated on the Vector engine (Zacc += Et per
    k-tile), partition-reduced once at the end with a ones-matmul.
  - Mean over q: W[d, q] stays PSUM-resident (3 banks), divided by Z via a
    broadcast row, one AllReduce of [128, 1], replicated MLP head.
  - Collectives: h (between layers) in 2 chunks, K^T/V (before attention) in
    4 chunks, all overlapped with compute; k-tiles consumed in arrival order.
"""

import math
import os

import numpy as np
import ml_dtypes

try:
    import scipy.sparse as _scipy_sparse
except ImportError:
    _scipy_sparse = None

import concourse.bass as bass
import concourse.bacc as bacc
import concourse.tile as tile
from concourse import mybir
from concourse.bass_utils import run_bass_kernel_spmd
from concourse.masks import make_identity

NCORES = 8
NREAL = 10000
NP = 10240           # padded node count
ND = NP // NCORES    # 1280 nodes per core
NT = ND // 128       # 10 tiles of 128 per core
SB = NP // 128       # 80 src blocks globally
D = 128
INV = 1.0 / math.sqrt(128.0)

BF16 = mybir.dt.bfloat16
FP8 = mybir.dt.float8e4
F32 = mybir.dt.float32

NP_BF16 = mybir.dt.np(BF16)
NP_FP8 = mybir.dt.np(FP8)

_NC_CACHE = {}

RELU = mybir.ActivationFunctionType.Relu
IDENT = mybir.ActivationFunctionType.Identity
EXP = mybir.ActivationFunctionType.Exp
ADD = mybir.AluOpType.add
MULT = mybir.AluOpType.mult
SL3 = ((0, 512), (512, 1024), (1024, 1280))
QA = ((0, 512), (512, 1024))   # score q-chunk A (1024 wide)
QB = (1024, 1280)              # score q-chunk B (256 wide)


def _build(phase=9):
    nc = bacc.Bacc("TRN2", target_bir_lowering=False, debug=False, num_devices=NCORES)

    A_in = nc.dram_tensor("a_cnt", [128, SB, ND], FP8, kind="ExternalInput")
    xt_in = nc.dram_tensor("x_tiled", [128, SB, D], BF16, kind="ExternalInput")
    xTm_in = nc.dram_tensor("xT_mine", [128, ND], BF16, kind="ExternalInput")
    qm128_in = nc.dram_tensor("qmask128", [128, ND], F32, kind="ExternalInput")
    names_bf = ["w1r", "w1l", "w2r", "w2l", "wq", "wk", "wv"]
    ins_bf = {n: nc.dram_tensor(n, [D, D], BF16, kind="ExternalInput") for n in names_bf}
    ins_f32 = {
        "b1": nc.dram_tensor("b1", [D, 1], F32, kind="ExternalInput"),
        "b2": nc.dram_tensor("b2", [D, 1], F32, kind="ExternalInput"),
        "qgv": nc.dram_tensor("qgv", [D, 1], F32, kind="ExternalInput"),
        "vgv": nc.dram_tensor("vgv", [D, 1], F32, kind="ExternalInput"),
        "negv": nc.dram_tensor("negv", [D, 1], F32, kind="ExternalInput"),
        "negbeta": nc.dram_tensor("negbeta", [D, 1], F32, kind="ExternalInput"),
        "wo": nc.dram_tensor("wo", [D, D], F32, kind="ExternalInput"),
        "wf1": nc.dram_tensor("wf1", [D, 64], F32, kind="ExternalInput"),
        "wf2": nc.dram_tensor("wf2", [64, 32], F32, kind="ExternalInput"),
        "wf3": nc.dram_tensor("wf3", [32, D], F32, kind="ExternalInput"),
        "bo": nc.dram_tensor("bo", [D, 1], F32, kind="ExternalInput"),
        "bf1": nc.dram_tensor("bf1", [64, 1], F32, kind="ExternalInput"),
        "bf2": nc.dram_tensor("bf2", [32, 1], F32, kind="ExternalInput"),
        "bf3": nc.dram_tensor("bf3", [D, 1], F32, kind="ExternalInput"),
    }
    out_t = nc.dram_tensor("out", [1, D], F32, kind="ExternalOutput")
    rg = [list(range(NCORES))]

    with tile.TileContext(nc) as tc:
        with (
            tc.tile_pool(name="dram", bufs=1, space="DRAM") as dram,
            tc.tile_pool(name="const", bufs=1) as cp,
            tc.tile_pool(name="live", bufs=1) as lp,
        ):
            dumb = dram.tile([128, 640], BF16)
            dumf = dram.tile([NCORES, 128, 640], BF16, addr_space="Shared")
            hb = dram.tile([128, ND], FP8)
            hfull = dram.tile([NCORES, 128, ND], FP8, addr_space="Shared")
            kvds = [dram.tile([128, 1024], BF16, name="kvd0"),
                    dram.tile([128, 1536], BF16, name="kvd1")]
            kvfulls = [dram.tile([NCORES, 128, 1024], BF16,
                                 addr_space="Shared", name="kvfull0"),
                       dram.tile([NCORES, 128, 1536], BF16,
                                 addr_space="Shared", name="kvfull1")]
            accb = dram.tile([128, 1], F32)
            accf = dram.tile([NCORES, 128, 1], F32, addr_space="Shared")

            # warm-up AllGather FIRST: its input DMA must not queue behind the
            # big loads, and the first collective pays ~50us of ncfw cold-start
            dums = cp.tile([128, 640], BF16)
            nc.gpsimd.memset(dums[:], 0)
            nc.sync.dma_start(out=dumb[:], in_=dums[:])
            warm_cc = nc.gpsimd.collective_compute(
                "AllGather", mybir.AluOpType.bypass, replica_groups=rg,
                ins=[dumb[:].opt()], outs=[dumf[:].opt()],
            )

            def cload(dram_t, shape, dtype):
                t = cp.tile(shape, dtype, tag=f"c_{dram_t.name}")
                nc.sync.dma_start(out=t[:], in_=dram_t[:])
                return t

            w1r = cload(ins_bf["w1r"], [D, D], BF16)
            w1l = cload(ins_bf["w1l"], [D, D], BF16)
            w2r = cload(ins_bf["w2r"], [D, D], BF16)
            w2l = cload(ins_bf["w2l"], [D, D], BF16)
            wq = cload(ins_bf["wq"], [D, D], BF16)
            wk = cload(ins_bf["wk"], [D, D], BF16)
            wv = cload(ins_bf["wv"], [D, D], BF16)
            b1 = cload(ins_f32["b1"], [D, 1], F32)
            b2 = cload(ins_f32["b2"], [D, 1], F32)
            qgv = cload(ins_f32["qgv"], [D, 1], F32)
            vgv = cload(ins_f32["vgv"], [D, 1], F32)
            negv = cload(ins_f32["negv"], [D, 1], F32)
            negbeta = cload(ins_f32["negbeta"], [D, 1], F32)
            wo = cload(ins_f32["wo"], [D, D], F32)
            wf1 = cload(ins_f32["wf1"], [D, 64], F32)
            wf2 = cload(ins_f32["wf2"], [64, 32], F32)
            wf3 = cload(ins_f32["wf3"], [32, D], F32)
            bo = cload(ins_f32["bo"], [D, 1], F32)
            bf1 = cload(ins_f32["bf1"], [64, 1], F32)
            bf2 = cload(ins_f32["bf2"], [32, 1], F32)
            bf3 = cload(ins_f32["bf3"], [D, 1], F32)
            xTm = cload(xTm_in, [128, ND], BF16)
            ident = cp.tile([128, 128], F32)
            make_identity(nc, ident[:])
            ones1 = cp.tile([1, 128], F32)
            nc.gpsimd.memset(ones1[:], 1.0)
            onesq = cp.tile([128, 128], F32)
            nc.gpsimd.memset(onesq[:], 1.0)
            qm128 = cload(qm128_in, [128, ND], F32)

            hT = lp.tile([128, ND], BF16)
            neT = lp.tile([128, ND], BF16)
            htiled = lp.tile([128, NT, D], BF16)
            kv = lp.tile([128, 2 * ND], BF16)
            QT = lp.tile([128, ND], BF16)

            # ================= message passing =================
            with (
                tc.tile_pool(name="sbL", bufs=1) as sl,
                tc.tile_pool(name="psL", bufs=1, space="PSUM") as psl,
                tc.tile_pool(name="kvq", bufs=2, space="PSUM") as kvqp,
            ):
                xt = sl.tile([128, SB, D], BF16)
                for j in range(4):
                    nc.sync.dma_start(out=xt[:, 20 * j : 20 * (j + 1), :],
                                      in_=xt_in[:, 20 * j : 20 * (j + 1), :])
                asb = sl.tile([128, SB, ND], FP8)
                for j in range(16):
                    nc.sync.dma_start(
                        out=asb[:, 5 * j : 5 * (j + 1), :],
                        in_=A_in[:, 5 * j : 5 * (j + 1), :],
                    )

                # ----- layer 1 -----
                z1 = psl.tile([128, ND], F32, tag="z", space="PSUM")
                for lo, hi in SL3:
                    nc.tensor.matmul(out=z1[:, lo:hi], lhsT=w1r[:],
                                     rhs=xTm[:, lo:hi], start=True, stop=False)
                agg_ps = psl.tile([128, ND], F32, tag="agg", space="PSUM")
                for s in range(SB):
                    for lo, hi in SL3:
                        nc.tensor.matmul(
                            out=agg_ps[:, lo:hi], lhsT=xt[:, s, :],
                            rhs=asb[:, s, lo:hi],
                            start=(s == 0), stop=(s == SB - 1),
                        )
                agg1 = sl.tile([128, ND], BF16, tag="agg1")
                for lo, hi in SL3:
                    nc.vector.tensor_copy(out=agg1[:, lo:hi], in_=agg_ps[:, lo:hi])
                for lo, hi in SL3:
                    nc.tensor.matmul(out=z1[:, lo:hi], lhsT=w1l[:],
                                     rhs=agg1[:, lo:hi], start=False, stop=True)
                    nc.scalar.activation(hT[:, lo:hi], z1[:, lo:hi], RELU,
                                         bias=b1[:])
                nc.sync.dma_start_transpose(out=htiled[:], in_=hT[:])
                ht8 = sl.tile([128, NT, D], FP8, tag="ht8")
                nc.vector.tensor_copy(
                    out=ht8[:].rearrange("p t d -> p (t d)"),
                    in_=htiled[:].rearrange("p t d -> p (t d)"))
                nc.sync.dma_start(
                    out=hb[:], in_=ht8[:].rearrange("p t d -> p (t d)"))
                nc.gpsimd.collective_compute(
                    "AllGather", mybir.AluOpType.bypass, replica_groups=rg,
                    ins=[hb[:].opt()], outs=[hfull[:].opt()],
                )

                # ----- layer 2: column-chunked, DoubleRow fp8 aggregation,
                # K/V computed per chunk so the kv gathers launch early -----
                z2 = psl.tile([128, ND], F32, tag="z", space="PSUM")
                for lo, hi in SL3:
                    nc.tensor.matmul(out=z2[:, lo:hi], lhsT=w2r[:],
                                     rhs=hT[:, lo:hi], start=True, stop=False)

                hts = sl.tile([128, NCORES, NT, D], FP8, tag="hts")
                for c in range(NCORES):
                    nc.sync.dma_start(out=hts[:, c, :, :], in_=hfull[c])

                pairs = [(c, u) for c in range(NCORES) for u in range(NT // 2)]
                for ci, (lo, hi) in enumerate(SL3):
                    w = hi - lo
                    aggc_ps = psl.tile([128, 512], F32, tag="agg", space="PSUM")
                    for si, (c, u) in enumerate(pairs):
                        s = c * NT + 2 * u
                        nc.tensor.matmul(
                            out=aggc_ps[:, 0:w],
                            lhsT=hts[:, c, 2 * u : 2 * u + 2, :],
                            rhs=asb[:, s : s + 2, lo:hi],
                            start=(si == 0), stop=(si == len(pairs) - 1),
                            perf_mode=mybir.MatmulPerfMode.DoubleRow,
                        )
                    agg2c = sl.tile([128, 512], BF16, tag="agg2c")
                    nc.vector.tensor_copy(out=agg2c[:, 0:w], in_=aggc_ps[:, 0:w])
                    nc.tensor.matmul(out=z2[:, lo:hi], lhsT=w2l[:],
                                     rhs=agg2c[:, 0:w], start=False, stop=True)
                    nc.scalar.activation(neT[:, lo:hi], z2[:, lo:hi], IDENT,
                                         bias=b2[:])
                    # K'^T chunk = WK^T neT - v  (no bK: constant-in-k shifts
                    # cancel; -v applies the fitted per-q stabilizer)
                    kps = kvqp.tile([128, 512], F32, tag="kvq", space="PSUM")
                    nc.tensor.matmul(out=kps[:, 0:w], lhsT=wk[:],
                                     rhs=neT[:, lo:hi], start=True, stop=True)
                    nc.scalar.activation(kv[:, lo:hi], kps[:, 0:w],
                                         IDENT, bias=negv[:])
                    # V tiled [k, d] for the 128-blocks inside this chunk
                    vps = kvqp.tile([128, 512], F32, tag="kvq", space="PSUM")
                    for j, b in enumerate(range(lo // 128, hi // 128)):
                        nc.tensor.matmul(
                            out=vps[:, 128 * j : 128 * (j + 1)],
                            lhsT=neT[:, 128 * b : 128 * (b + 1)],
                            rhs=wv[:], start=True, stop=True,
                        )
                    nc.vector.tensor_copy(out=kv[:, ND + lo : ND + hi],
                                          in_=vps[:, 0:w])
                    if ci == 0:
                        # K,V for k-tiles 0-3 complete: launch chunk-1 gather
                        nc.sync.dma_start(out=kvds[0][:, 0:512], in_=kv[:, 0:512])
                        nc.sync.dma_start(out=kvds[0][:, 512:1024],
                                          in_=kv[:, ND : ND + 512])
                        nc.gpsimd.collective_compute(
                            "AllGather", mybir.AluOpType.bypass,
                            replica_groups=rg,
                            ins=[kvds[0][:].opt()], outs=[kvfulls[0][:].opt()],
                        )
                nc.sync.dma_start(out=kvds[1][:, 0:768], in_=kv[:, 512:1280])
                nc.sync.dma_start(out=kvds[1][:, 768:1536],
                                  in_=kv[:, ND + 512 : ND + 1280])
                nc.gpsimd.collective_compute(
                    "AllGather", mybir.AluOpType.bypass, replica_groups=rg,
                    ins=[kvds[1][:].opt()], outs=[kvfulls[1][:].opt()],
                )
                # Q while the gathers fly
                for lo, hi in SL3:
                    qps = kvqp.tile([128, 512], F32, tag="kvq", space="PSUM")
                    nc.tensor.matmul(out=qps[:, 0 : hi - lo], lhsT=wq[:],
                                     rhs=neT[:, lo:hi], start=True, stop=True)
                    nc.scalar.activation(QT[:, lo:hi], qps[:, 0 : hi - lo],
                                         IDENT, bias=qgv[:])

            # ================= attention =================
            with tc.tile_pool(name="sbA", bufs=1) as sa:
                Zacc = sa.tile([128, 2, ND], F32)
                kvf = sa.tile([128, NCORES, 2 * ND], BF16)

                for r in range(NCORES):
                    nc.sync.dma_start(out=kvf[:, r, 0:512],
                                      in_=kvfulls[0][r][:, 0:512])
                    nc.sync.dma_start(out=kvf[:, r, ND : ND + 512],
                                      in_=kvfulls[0][r][:, 512:1024])
                for r in range(NCORES):
                    nc.sync.dma_start(out=kvf[:, r, 512:1280],
                                      in_=kvfulls[1][r][:, 0:768])
                    nc.sync.dma_start(out=kvf[:, r, ND + 512 : ND + 1280],
                                      in_=kvfulls[1][r][:, 768:1536])

                # all 80 k-tiles, chunk-1 tiles first, paired for one exp per
                # two tiles; the final pair is ((7,8),(7,9)) = the pad tiles
                korder = ([(r, t) for r in range(NCORES) for t in range(4)]
                          + [(r, t) for r in range(NCORES) for t in range(4, NT)])
                kpairs = [(korder[2 * i], korder[2 * i + 1])
                          for i in range(len(korder) // 2)]
                halves = [(p, h) for p in range(len(kpairs)) for h in (0, 1)
                          if kpairs[p][h] != (7, 9)]
                first_half, last_half = halves[0], halves[-1]

                with tc.tile_pool(name="wtp", bufs=1, space="PSUM") as wtp:
                    wta = wtp.tile([128, 1024], F32, tag="wta", space="PSUM")
                    wtb = wtp.tile([128, 256], F32, tag="wtb", space="PSUM")

                    with (
                        tc.tile_pool(name="stp", bufs=1, space="PSUM") as stp,
                        tc.tile_pool(name="ep", bufs=3) as ep,
                    ):
                        pending = None

                        def emit_pv(pv):
                            et, pi, pair = pv
                            for h in (0, 1):
                                r, t = pair[h]
                                if (r, t) == (7, 9):
                                    continue
                                vt = kvf[:, r, ND + 128 * t : ND + 128 * (t + 1)]
                                first = (pi, h) == first_half
                                last = (pi, h) == last_half
                                for lo, hi in QA:
                                    nc.tensor.matmul(
                                        out=wta[:, lo:hi], lhsT=vt,
                                        rhs=et[:, h, lo:hi],
                                        start=first, stop=last,
                                    )
                                nc.tensor.matmul(
                                    out=wtb[:], lhsT=vt,
                                    rhs=et[:, h, 1024:1280],
                                    start=first, stop=last,
                                )

                        for pi, pair in enumerate(kpairs):
                            sta = stp.tile([128, 2, 1024], F32, tag="sta",
                                           space="PSUM")
                            stb = stp.tile([128, 2, 256], F32, tag="stb",
                                           space="PSUM")
                            for h in (0, 1):
                                r, t = pair[h]
                                if (r, t) == (7, 9):
                                    continue
                                kt = kvf[:, r, 128 * t : 128 * (t + 1)]
                                for lo, hi in QA:
                                    nc.tensor.matmul(out=sta[:, h, lo:hi],
                                                     lhsT=kt, rhs=QT[:, lo:hi],
                                                     start=True, stop=True)
                                nc.tensor.matmul(out=stb[:, h, :], lhsT=kt,
                                                 rhs=QT[:, QB[0]:QB[1]],
                                                 start=True, stop=True)
                            et = ep.tile([128, 2, ND], BF16, tag="et")
                            if pair[1] == (7, 9):
                                # pad pair: zero everything, exp only the 16
                                # real rows of half 0 = global nodes 9984..10000
                                nc.gpsimd.memset(et[:], 0)
                                nc.scalar.activation(et[0:16, 0, 0:1024],
                                                     sta[0:16, 0, :], EXP,
                                                     scale=INV,
                                                     bias=negbeta[0:16])
                                nc.scalar.activation(et[0:16, 0, 1024:1280],
                                                     stb[0:16, 0, :], EXP,
                                                     scale=INV,
                                                     bias=negbeta[0:16])
                            else:
                                # per-half exp: the next pair's score matmuls
                                # for half 0 can start as soon as half 0 is
                                # read, keeping ACT off the PE critical path
                                nc.scalar.activation(et[:, 0, 0:1024],
                                                     sta[:, 0, :], EXP,
                                                     scale=INV, bias=negbeta[:])
                                nc.scalar.activation(et[:, 1, 0:1024],
                                                     sta[:, 1, :], EXP,
                                                     scale=INV, bias=negbeta[:])
                                nc.scalar.activation(et[:, :, 1024:1280],
                                                     stb[:], EXP,
                                                     scale=INV, bias=negbeta[:])
                            # Z accumulation, two-level: bf16 sum of two pairs,
                            # f32 master accumulate every 2 pairs
                            if pi % 2 == 0:
                                held = et
                            else:
                                esum = ep.tile([128, 2, ND], BF16, tag="esum")
                                nc.vector.tensor_tensor(out=esum[:],
                                                        in0=held[:],
                                                        in1=et[:], op=ADD)
                                if pi == 1:
                                    nc.vector.tensor_copy(out=Zacc[:],
                                                          in_=esum[:])
                                else:
                                    nc.vector.tensor_tensor(out=Zacc[:],
                                                            in0=Zacc[:],
                                                            in1=esum[:], op=ADD)
                            if pending is not None:
                                emit_pv(pending)
                            pending = (et, pi, pair)
                        emit_pv(pending)

                    # ---------- epilogue ----------
                    # Z broadcast: ones[128,128]^T @ Zacc puts sum_k in every
                    # output partition, keeping all DVE work 128 lanes wide
                    with tc.tile_pool(name="ez", bufs=1, space="PSUM") as ezp:
                        zbp = ezp.tile([128, ND], F32, tag="ez", space="PSUM")
                        for lo, hi in SL3:
                            for h in (0, 1):
                                nc.tensor.matmul(out=zbp[:, lo:hi],
                                                 lhsT=onesq[:],
                                                 rhs=Zacc[:, h, lo:hi],
                                                 start=(h == 0), stop=(h == 1))
                        zb = sa.tile([128, ND], F32, tag="zb")
                        nc.vector.tensor_scalar_add(out=zb[:], in0=zbp[:],
                                                    scalar1=1e-30)
                        rz = sa.tile([128, ND], F32, tag="rz")
                        nc.vector.reciprocal_approx_fast(out=rz[:], in_=zb[:])
                        rb = sa.tile([128, ND], F32, tag="rb")
                        nc.vector.tensor_tensor(out=rb[:], in0=rz[:],
                                                in1=qm128[:], op=MULT)
                        wn = sa.tile([128, ND], F32, tag="wn")
                        nc.vector.tensor_tensor(out=wn[:, 0:1024], in0=wta[:],
                                                in1=rb[:, 0:1024], op=MULT)
                        nc.vector.tensor_tensor(out=wn[:, 1024:1280], in0=wtb[:],
                                                in1=rb[:, 1024:1280], op=MULT)
                        acc = sa.tile([128, 1], F32, tag="acc")
                        nc.vector.reduce_sum(acc[:], wn[:],
                                             axis=mybir.AxisListType.X)
                        nc.sync.dma_start(out=accb[:], in_=acc[:])
                        # AllGather + local sum beats AllReduce's latency floor
                        nc.gpsimd.collective_compute(
                            "AllGather", mybir.AluOpType.bypass, replica_groups=rg,
                            ins=[accb[:].opt()], outs=[accf[:].opt()],
                        )
                        accg = sa.tile([128, NCORES], F32, tag="accg")
                        for r in range(NCORES):
                            nc.sync.dma_start(out=accg[:, r : r + 1], in_=accf[r])
                        acc2 = sa.tile([128, 1], F32, tag="acc2")
                        nc.vector.reduce_sum(acc2[:], accg[:],
                                             axis=mybir.AxisListType.X)
                        aggc = sa.tile([128, 1], F32, tag="aggc")
                        nc.scalar.activation(aggc[:], acc2[:], IDENT,
                                             scale=1.0 / NREAL, bias=vgv[:])

                        # ---------- tiny MLP head (replicated) ----------
                        hd = ezp.tile([128, 512], F32, tag="hd", space="PSUM")
                        nc.tensor.matmul(out=hd[:, 0:1], lhsT=wo[:], rhs=aggc[:],
                                         start=True, stop=True)
                        state = sa.tile([128, 1], F32, tag="state")
                        nc.scalar.activation(state[:], hd[:, 0:1], IDENT, bias=bo[:])
                        hd2 = ezp.tile([128, 512], F32, tag="hd", space="PSUM")
                        nc.tensor.matmul(out=hd2[:64, 0:1], lhsT=wf1[:], rhs=state[:],
                                         start=True, stop=True)
                        x1 = sa.tile([64, 1], F32, tag="x1")
                        nc.scalar.activation(x1[:], hd2[:64, 0:1], RELU, bias=bf1[:])
                        hd3 = ezp.tile([128, 512], F32, tag="hd", space="PSUM")
                        nc.tensor.matmul(out=hd3[:32, 0:1], lhsT=wf2[:], rhs=x1[:],
                                         start=True, stop=True)
                        x2 = sa.tile([32, 1], F32, tag="x2")
                        nc.scalar.activation(x2[:], hd3[:32, 0:1], RELU, bias=bf2[:])
                        hd4 = ezp.tile([128, 512], F32, tag="hd", space="PSUM")
                        nc.tensor.matmul(out=hd4[:, 0:1], lhsT=wf3[:], rhs=x2[:],
                                         start=True, stop=True)
                        lg = sa.tile([128, 1], F32, tag="lg")
                        nc.scalar.activation(lg[:], hd4[:, 0:1], IDENT, bias=bf3[:])
                        hd5 = ezp.tile([128, 512], F32, tag="hd", space="PSUM")
                        nc.tensor.transpose(out=hd5[:1, 0:128], in_=lg[:],
                                            identity=ident[:])
                        er = sa.tile([1, 128], F32, tag="er")
                        zf = sa.tile([1, 1], F32, tag="zf")
                        nc.scalar.activation(er[:], hd5[:1, 0:128], EXP,
                                             accum_out=zf[:])
                        rzf = sa.tile([1, 1], F32, tag="rzf")
                        nc.vector.reciprocal(rzf[:], zf[:])
                        orow = sa.tile([1, 128], F32, tag="orow")
                        nc.vector.tensor_scalar(out=orow[:], in0=er[:],
                                                scalar1=rzf[:], scalar2=None,
                                                op0=MULT)
                        nc.sync.dma_start(out=out_t[:], in_=orow[:])

    nc.compile()
    return nc


def _get_nc():
    phase = int(os.environ.get("K_PHASE", "9"))
    key = ("nc", phase)
    if key not in _NC_CACHE:
        _NC_CACHE[key] = _build(phase)
    return _NC_CACHE[key]


def _prep_in_maps(inputs):
    f32 = np.float32
    x = np.asarray(inputs["node_features"], f32)
    g = np.asarray(inputs["global_info"], f32)
    ei = np.asarray(inputs["edge_index"])
    src = np.asarray(ei[0], np.int64)
    dst = np.asarray(ei[1], np.int64)

    xp = np.zeros((NP, D), f32)
    xp[:NREAL] = x
    xb = xp.astype(NP_BF16)
    x_tiled = np.ascontiguousarray(xb.reshape(SB, 128, D).transpose(1, 0, 2))

    qgv = (np.asarray(inputs["bQ"], f32)
           + (g @ np.asarray(inputs["WQg"], f32))[0]
           + np.asarray(inputs["bQg"], f32))
    vgv = (np.asarray(inputs["bV"], f32)
           + (g @ np.asarray(inputs["WVg"], f32))[0]
           + np.asarray(inputs["bVg"], f32))

    # host-side shift fit: c_q ~= Q_q . u + beta tracks the per-row max of
    # the (unshifted) scores so exp stays in fp32/bf16 range on device
    if _scipy_sparse is not None:
        adj = _scipy_sparse.csr_matrix(
            (np.ones(src.shape[0], f32), (dst, src)), shape=(NREAL, NREAL))
        segsum = lambda m: adj @ m
    else:
        order = np.argsort(dst, kind="stable")
        dsort = dst[order]
        ssort = src[order]
        starts = np.flatnonzero(np.r_[True, dsort[1:] != dsort[:-1]])
        uniq = dsort[starts]

        def segsum(m):
            out = np.zeros((NREAL, m.shape[1]), f32)
            out[uniq] = np.add.reduceat(m[ssort], starts, axis=0)
            return out

    W1r = np.asarray(inputs["W1_root"], f32)
    W1l = np.asarray(inputs["W1_rel"], f32)
    W2r = np.asarray(inputs["W2_root"], f32)
    W2l = np.asarray(inputs["W2_rel"], f32)
    h_h = np.maximum(x @ W1r + segsum(x) @ W1l + np.asarray(inputs["b1"], f32), 0)
    ne_h = h_h @ W2r + segsum(h_h) @ W2l + np.asarray(inputs["b2"], f32)
    Q_h = ne_h @ np.asarray(inputs["WQ"], f32) + qgv
    K_h = ne_h @ np.asarray(inputs["WK"], f32)          # device K has no bias
    rowmax_sub = ((Q_h @ K_h[::16].T) * INV).max(axis=1)
    Afit = np.hstack([Q_h, np.ones((NREAL, 1), f32)]).astype(np.float64)
    yfit = rowmax_sub.astype(np.float64)
    AtA = Afit.T @ Afit
    lam = 1e-4 * np.mean(np.diag(AtA)[:D])
    sol = np.linalg.solve(AtA + lam * np.eye(D + 1), Afit.T @ yfit)
    resid = rowmax_sub - (Afit @ sol).astype(f32)
    delta = float(resid.max()) - 8.0
    u = sol[:D].astype(f32)
    beta = float(sol[D]) + delta
    negv = (-(u / INV)).reshape(D, 1).astype(f32)
    negbeta = np.full((D, 1), -beta, f32)

    def bf(name):
        return np.ascontiguousarray(np.asarray(inputs[name], f32).astype(NP_BF16))

    shared = {
        "w1r": bf("W1_root"), "w1l": bf("W1_rel"),
        "w2r": bf("W2_root"), "w2l": bf("W2_rel"),
        "wq": bf("WQ"), "wk": bf("WK"), "wv": bf("WV"),
        "b1": np.asarray(inputs["b1"], f32).reshape(D, 1),
        "b2": np.asarray(inputs["b2"], f32).reshape(D, 1),
        "qgv": qgv.reshape(D, 1).copy(), "vgv": vgv.reshape(D, 1).copy(),
        "negv": negv, "negbeta": negbeta,
        "wo": np.asarray(inputs["Wo"], f32),
        "wf1": np.asarray(inputs["Wfc1"], f32),
        "wf2": np.asarray(inputs["Wfc2"], f32),
        "wf3": np.asarray(inputs["Wfc3"], f32),
        "bo": np.asarray(inputs["bo"], f32).reshape(D, 1),
        "bf1": np.asarray(inputs["bfc1"], f32).reshape(64, 1),
        "bf2": np.asarray(inputs["bfc2"], f32).reshape(32, 1),
        "bf3": np.asarray(inputs["bfc3"], f32).reshape(D, 1),
        "x_tiled": x_tiled,
    }

    core_of = dst // ND
    in_maps = []
    nodes = np.arange(NP)
    for c in range(NCORES):
        m = core_of == c
        A = np.zeros((NP, ND), f32)
        np.add.at(A, (src[m], dst[m] - ND * c), 1.0)
        Ac = np.ascontiguousarray(
            A.reshape(SB, 128, ND).transpose(1, 0, 2)
        ).astype(NP_FP8)
        xTmc = np.ascontiguousarray(xb[ND * c : ND * (c + 1)].T)
        qm = (nodes[ND * c : ND * (c + 1)] < NREAL).astype(f32)
        qm128 = np.ascontiguousarray(np.broadcast_to(qm.reshape(1, ND), (128, ND)))
        in_maps.append({**shared, "a_cnt": Ac, "xT_mine": xTmc,
                        "qmask128": qm128})
    return in_maps


def kernel(**inputs):
    nc = _get_nc()
    in_maps = _prep_in_maps(inputs)
    res = run_bass_kernel_spmd(nc, in_maps, core_ids=list(range(NCORES)))
    return np.asarray(res.results[0]["out"], np.float32)


# revision 70
# speedup vs baseline: 1.1696x; 1.1696x over previous
"""GNN message passing + global softmax attention + MLP head on 8 TRN2 NeuronCores.

v2 strategy (node/dst-sharded SPMD, single program, rank enters via data):
  - GraphConv aggregation as block-dense adjacency matmuls (fp8 count matrix,
    SBUF-resident across both layers), same as v1.
  - Transposed-score attention: S^T[k, q] = K'^T Q is computed with k on
    partitions so the exp'd tiles feed the PV matmul directly (lhsT = V tile,
    rhs = exp tile) -- no DMA transpose of the score tiles at all.
  - Numerical range without a per-row max pass: host fits a linear model
    c_q ~= Q_q . u + beta to the (subsampled) row max of the scores, then
    folds -sqrt(128)*u into the K bias (per-q shift is constant along k, so
    softmax is mathematically unchanged) and -beta into the exp bias.
  - Softmax denominator accumulated on the Vector engine (Zacc += Et per
    k-tile), partition-reduced once at the end with a ones-matmul.
  - Mean over q: W[d, q] stays PSUM-resident (3 banks), divided by Z via a
    broadcast row, one AllReduce of [128, 1], replicated MLP head.
  - Collectives: h (between layers) in 2 chunks, K^T/V (before attention) in
    4 chunks, all overlapped with compute; k-tiles consumed in arrival order.
"""

import math
import os

import numpy as np
import ml_dtypes

try:
    import scipy.sparse as _scipy_sparse
except ImportError:
    _scipy_sparse = None

import concourse.bass as bass
import concourse.bacc as bacc
import concourse.tile as tile
from concourse import mybir
from concourse.bass_utils import run_bass_kernel_spmd
from concourse.masks import make_identity

NCORES = 8
NREAL = 10000
NP = 10240           # padded node count
ND = NP // NCORES    # 1280 nodes per core
NT = ND // 128       # 10 tiles of 128 per core
SB = NP // 128       # 80 src blocks globally
D = 128
INV = 1.0 / math.sqrt(128.0)

BF16 = mybir.dt.bfloat16
FP8 = mybir.dt.float8e4
F32 = mybir.dt.float32

NP_BF16 = mybir.dt.np(BF16)
NP_FP8 = mybir.dt.np(FP8)

_NC_CACHE = {}

RELU = mybir.ActivationFunctionType.Relu
IDENT = mybir.ActivationFunctionType.Identity
EXP = mybir.ActivationFunctionType.Exp
ADD = mybir.AluOpType.add
MULT = mybir.AluOpType.mult
SL3 = ((0, 512), (512, 1024), (1024, 1280))
QA = ((0, 512), (512, 1024))   # score q-chunk A (1024 wide)
QB = (1024, 1280)              # score q-chunk B (256 wide)


def _build(phase=9):
    nc = bacc.Bacc("TRN2", target_bir_lowering=False, debug=False, num_devices=NCORES)

    A_in = nc.dram_tensor("a_cnt", [128, SB, ND], FP8, kind="ExternalInput")
    xt_in = nc.dram_tensor("x_tiled", [128, SB, D], BF16, kind="ExternalInput")
    xTm_in = nc.dram_tensor("xT_mine", [128, ND], BF16, kind="ExternalInput")
    qm128_in = nc.dram_tensor("qmask128", [128, ND], F32, kind="ExternalInput")
    names_bf = ["w1r", "w1l", "w2r", "w2l", "wq", "wk", "wv"]
    ins_bf = {n: nc.dram_tensor(n, [D, D], BF16, kind="ExternalInput") for n in names_bf}
    ins_f32 = {
        "b1": nc.dram_tensor("b1", [D, 1], F32, kind="ExternalInput"),
        "b2": nc.dram_tensor("b2", [D, 1], F32, kind="ExternalInput"),
        "qgv": nc.dram_tensor("qgv", [D, 1], F32, kind="ExternalInput"),
        "vgv": nc.dram_tensor("vgv", [D, 1], F32, kind="ExternalInput"),
        "negv": nc.dram_tensor("negv", [D, 1], F32, kind="ExternalInput"),
        "negbeta": nc.dram_tensor("negbeta", [D, 1], F32, kind="ExternalInput"),
        "wo": nc.dram_tensor("wo", [D, D], F32, kind="ExternalInput"),
        "wf1": nc.dram_tensor("wf1", [D, 64], F32, kind="ExternalInput"),
        "wf2": nc.dram_tensor("wf2", [64, 32], F32, kind="ExternalInput"),
        "wf3": nc.dram_tensor("wf3", [32, D], F32, kind="ExternalInput"),
        "bo": nc.dram_tensor("bo", [D, 1], F32, kind="ExternalInput"),
        "bf1": nc.dram_tensor("bf1", [64, 1], F32, kind="ExternalInput"),
        "bf2": nc.dram_tensor("bf2", [32, 1], F32, kind="ExternalInput"),
        "bf3": nc.dram_tensor("bf3", [D, 1], F32, kind="ExternalInput"),
    }
    out_t = nc.dram_tensor("out", [1, D], F32, kind="ExternalOutput")
    rg = [list(range(NCORES))]

    with tile.TileContext(nc) as tc:
        with (
            tc.tile_pool(name="dram", bufs=1, space="DRAM") as dram,
            tc.tile_pool(name="const", bufs=1) as cp,
            tc.tile_pool(name="live", bufs=1) as lp,
        ):
            dumb = dram.tile([128, 640], BF16)
            dumf = dram.tile([NCORES, 128, 640], BF16, addr_space="Shared")
            hb = dram.tile([128, ND], FP8)
            hfull = dram.tile([NCORES, 128, ND], FP8, addr_space="Shared")
            kvds = [dram.tile([128, 1024], BF16, name="kvd0"),
                    dram.tile([128, 1536], BF16, name="kvd1")]
            kvfulls = [dram.tile([NCORES, 128, 1024], BF16,
                                 addr_space="Shared", name="kvfull0"),
                       dram.tile([NCORES, 128, 1536], BF16,
                                 addr_space="Shared", name="kvfull1")]
            accb = dram.tile([128, 1], F32)
            accf = dram.tile([NCORES, 128, 1], F32, addr_space="Shared")

            # warm-up AllGather FIRST: its input DMA must not queue behind the
            # big loads, and the first collective pays ~50us of ncfw cold-start
            dums = cp.tile([128, 640], BF16)
            nc.gpsimd.memset(dums[:], 0)
            nc.sync.dma_start(out=dumb[:], in_=dums[:])
            warm_cc = nc.gpsimd.collective_compute(
                "AllGather", mybir.AluOpType.bypass, replica_groups=rg,
                ins=[dumb[:].opt()], outs=[dumf[:].opt()],
            )

            def cload(dram_t, shape, dtype):
                t = cp.tile(shape, dtype, tag=f"c_{dram_t.name}")
                nc.sync.dma_start(out=t[:], in_=dram_t[:])
                return t

            w1r = cload(ins_bf["w1r"], [D, D], BF16)
            w1l = cload(ins_bf["w1l"], [D, D], BF16)
            w2r = cload(ins_bf["w2r"], [D, D], BF16)
            w2l = cload(ins_bf["w2l"], [D, D], BF16)
            wq = cload(ins_bf["wq"], [D, D], BF16)
            wk = cload(ins_bf["wk"], [D, D], BF16)
            wv = cload(ins_bf["wv"], [D, D], BF16)
            b1 = cload(ins_f32["b1"], [D, 1], F32)
            b2 = cload(ins_f32["b2"], [D, 1], F32)
            qgv = cload(ins_f32["qgv"], [D, 1], F32)
            vgv = cload(ins_f32["vgv"], [D, 1], F32)
            negv = cload(ins_f32["negv"], [D, 1], F32)
            negbeta = cload(ins_f32["negbeta"], [D, 1], F32)
            wo = cload(ins_f32["wo"], [D, D], F32)
            wf1 = cload(ins_f32["wf1"], [D, 64], F32)
            wf2 = cload(ins_f32["wf2"], [64, 32], F32)
            wf3 = cload(ins_f32["wf3"], [32, D], F32)
            bo = cload(ins_f32["bo"], [D, 1], F32)
            bf1 = cload(ins_f32["bf1"], [64, 1], F32)
            bf2 = cload(ins_f32["bf2"], [32, 1], F32)
            bf3 = cload(ins_f32["bf3"], [D, 1], F32)
            xTm = cload(xTm_in, [128, ND], BF16)
            ident = cp.tile([128, 128], F32)
            make_identity(nc, ident[:])
            ones1 = cp.tile([1, 128], F32)
            nc.gpsimd.memset(ones1[:], 1.0)
            onesq = cp.tile([128, 128], F32)
            nc.gpsimd.memset(onesq[:], 1.0)
            qm128 = cload(qm128_in, [128, ND], F32)

            hT = lp.tile([128, ND], BF16)
            neT = lp.tile([128, ND], BF16)
            htiled = lp.tile([128, NT, D], BF16)
            kv = lp.tile([128, 2 * ND], BF16)
            QT = lp.tile([128, ND], BF16)

            # ================= message passing =================
            with (
                tc.tile_pool(name="sbL", bufs=1) as sl,
                tc.tile_pool(name="psL", bufs=1, space="PSUM") as psl,
                tc.tile_pool(name="kvq", bufs=2, space="PSUM") as kvqp,
            ):
                xt = sl.tile([128, SB, D], BF16)
                for j in range(4):
                    nc.sync.dma_start(out=xt[:, 20 * j : 20 * (j + 1), :],
                                      in_=xt_in[:, 20 * j : 20 * (j + 1), :])
                asb = sl.tile([128, SB, ND], FP8)
                for j in range(16):
                    nc.sync.dma_start(
                        out=asb[:, 5 * j : 5 * (j + 1), :],
                        in_=A_in[:, 5 * j : 5 * (j + 1), :],
                    )

                # ----- layer 1 -----
                z1 = psl.tile([128, ND], F32, tag="z", space="PSUM")
                for lo, hi in SL3:
                    nc.tensor.matmul(out=z1[:, lo:hi], lhsT=w1r[:],
                                     rhs=xTm[:, lo:hi], start=True, stop=False)
                agg_ps = psl.tile([128, ND], F32, tag="agg", space="PSUM")
                for s in range(SB):
                    for lo, hi in SL3:
                        nc.tensor.matmul(
                            out=agg_ps[:, lo:hi], lhsT=xt[:, s, :],
                            rhs=asb[:, s, lo:hi],
                            start=(s == 0), stop=(s == SB - 1),
                        )
                agg1 = sl.tile([128, ND], BF16, tag="agg1")
                for lo, hi in SL3:
                    nc.vector.tensor_copy(out=agg1[:, lo:hi], in_=agg_ps[:, lo:hi])
                for lo, hi in SL3:
                    nc.tensor.matmul(out=z1[:, lo:hi], lhsT=w1l[:],
                                     rhs=agg1[:, lo:hi], start=False, stop=True)
                    nc.scalar.activation(hT[:, lo:hi], z1[:, lo:hi], RELU,
                                         bias=b1[:])
                nc.sync.dma_start_transpose(out=htiled[:], in_=hT[:])
                ht8 = sl.tile([128, NT, D], FP8, tag="ht8")
                nc.vector.tensor_copy(
                    out=ht8[:].rearrange("p t d -> p (t d)"),
                    in_=htiled[:].rearrange("p t d -> p (t d)"))
                nc.sync.dma_start(
                    out=hb[:], in_=ht8[:].rearrange("p t d -> p (t d)"))
                nc.gpsimd.collective_compute(
                    "AllGather", mybir.AluOpType.bypass, replica_groups=rg,
                    ins=[hb[:].opt()], outs=[hfull[:].opt()],
                )

                # ----- layer 2: column-chunked, DoubleRow fp8 aggregation,
                # K/V computed per chunk so the kv gathers launch early -----
                z2 = psl.tile([128, ND], F32, tag="z", space="PSUM")
                for lo, hi in SL3:
                    nc.tensor.matmul(out=z2[:, lo:hi], lhsT=w2r[:],
                                     rhs=hT[:, lo:hi], start=True, stop=False)

                hts = sl.tile([128, NCORES, NT, D], FP8, tag="hts")
                for c in range(NCORES):
                    nc.sync.dma_start(out=hts[:, c, :, :], in_=hfull[c])

                pairs = [(c, u) for c in range(NCORES) for u in range(NT // 2)]
                for ci, (lo, hi) in enumerate(SL3):
                    w = hi - lo
                    aggc_ps = psl.tile([128, 512], F32, tag="agg", space="PSUM")
                    for si, (c, u) in enumerate(pairs):
                        s = c * NT + 2 * u
                        nc.tensor.matmul(
                            out=aggc_ps[:, 0:w],
                            lhsT=hts[:, c, 2 * u : 2 * u + 2, :],
                            rhs=asb[:, s : s + 2, lo:hi],
                            start=(si == 0), stop=(si == len(pairs) - 1),
                            perf_mode=mybir.MatmulPerfMode.DoubleRow,
                        )
                    agg2c = sl.tile([128, 512], BF16, tag="agg2c")
                    nc.vector.tensor_copy(out=agg2c[:, 0:w], in_=aggc_ps[:, 0:w])
                    nc.tensor.matmul(out=z2[:, lo:hi], lhsT=w2l[:],
                                     rhs=agg2c[:, 0:w], start=False, stop=True)
                    nc.scalar.activation(neT[:, lo:hi], z2[:, lo:hi], IDENT,
                                         bias=b2[:])
                    # K'^T chunk = WK^T neT - v  (no bK: constant-in-k shifts
                    # cancel; -v applies the fitted per-q stabilizer)
                    kps = kvqp.tile([128, 512], F32, tag="kvq", space="PSUM")
                    nc.tensor.matmul(out=kps[:, 0:w], lhsT=wk[:],
                                     rhs=neT[:, lo:hi], start=True, stop=True)
                    nc.scalar.activation(kv[:, lo:hi], kps[:, 0:w],
                                         IDENT, bias=negv[:])
                    # V tiled [k, d] for the 128-blocks inside this chunk
                    vps = kvqp.tile([128, 512], F32, tag="kvq", space="PSUM")
                    for j, b in enumerate(range(lo // 128, hi // 128)):
                        nc.tensor.matmul(
                            out=vps[:, 128 * j : 128 * (j + 1)],
                            lhsT=neT[:, 128 * b : 128 * (b + 1)],
                            rhs=wv[:], start=True, stop=True,
                        )
                    nc.vector.tensor_copy(out=kv[:, ND + lo : ND + hi],
                                          in_=vps[:, 0:w])
                    if ci == 0:
                        # K,V for k-tiles 0-3 complete: launch chunk-1 gather
                        nc.sync.dma_start(out=kvds[0][:, 0:512], in_=kv[:, 0:512])
                        nc.sync.dma_start(out=kvds[0][:, 512:1024],
                                          in_=kv[:, ND : ND + 512])
                        nc.gpsimd.collective_compute(
                            "AllGather", mybir.AluOpType.bypass,
                            replica_groups=rg,
                            ins=[kvds[0][:].opt()], outs=[kvfulls[0][:].opt()],
                        )
                nc.sync.dma_start(out=kvds[1][:, 0:768], in_=kv[:, 512:1280])
                nc.sync.dma_start(out=kvds[1][:, 768:1536],
                                  in_=kv[:, ND + 512 : ND + 1280])
                nc.gpsimd.collective_compute(
                    "AllGather", mybir.AluOpType.bypass, replica_groups=rg,
                    ins=[kvds[1][:].opt()], outs=[kvfulls[1][:].opt()],
                )
                # Q while the gathers fly
                for lo, hi in SL3:
                    qps = kvqp.tile([128, 512], F32, tag="kvq", space="PSUM")
                    nc.tensor.matmul(out=qps[:, 0 : hi - lo], lhsT=wq[:],
                                     rhs=neT[:, lo:hi], start=True, stop=True)
                    nc.scalar.activation(QT[:, lo:hi], qps[:, 0 : hi - lo],
                                         IDENT, bias=qgv[:])

            # ================= attention =================
            with tc.tile_pool(name="sbA", bufs=1) as sa:
                Zacc = sa.tile([128, 2, 1024], F32)
                Zaccb = sa.tile([128, 2, 256], F32)
                kvf = sa.tile([128, NCORES, 2 * ND], BF16)

                for r in range(NCORES):
                    nc.sync.dma_start(out=kvf[:, r, 0:512],
                                      in_=kvfulls[0][r][:, 0:512])
                    nc.sync.dma_start(out=kvf[:, r, ND : ND + 512],
                                      in_=kvfulls[0][r][:, 512:1024])
                for r in range(NCORES):
                    nc.sync.dma_start(out=kvf[:, r, 512:1280],
                                      in_=kvfulls[1][r][:, 0:768])
                    nc.sync.dma_start(out=kvf[:, r, ND + 512 : ND + 1280],
                                      in_=kvfulls[1][r][:, 768:1536])

                # all 80 k-tiles, chunk-1 tiles first, paired for one exp per
                # two tiles; the final pair is ((7,8),(7,9)) = the pad tiles
                korder = ([(r, t) for r in range(NCORES) for t in range(4)]
                          + [(r, t) for r in range(NCORES) for t in range(4, NT)])
                kpairs = [(korder[2 * i], korder[2 * i + 1])
                          for i in range(len(korder) // 2)]
                halves = [(p, h) for p in range(len(kpairs)) for h in (0, 1)
                          if kpairs[p][h] != (7, 9)]
                first_half, last_half = halves[0], halves[-1]

                with tc.tile_pool(name="wtp", bufs=1, space="PSUM") as wtp:
                    wta = wtp.tile([128, 1024], F32, tag="wta", space="PSUM")
                    wtb = wtp.tile([128, 256], F32, tag="wtb", space="PSUM")

                    with (
                        tc.tile_pool(name="stp", bufs=1, space="PSUM") as stp,
                        tc.tile_pool(name="ep", bufs=3) as ep,
                    ):
                        pending = None

                        def emit_pv(pv):
                            eta, etb, pi, pair = pv
                            for h in (0, 1):
                                r, t = pair[h]
                                if (r, t) == (7, 9):
                                    continue
                                vt = kvf[:, r, ND + 128 * t : ND + 128 * (t + 1)]
                                first = (pi, h) == first_half
                                last = (pi, h) == last_half
                                for lo, hi in QA:
                                    nc.tensor.matmul(
                                        out=wta[:, lo:hi], lhsT=vt,
                                        rhs=eta[:, h, lo:hi],
                                        start=first, stop=last,
                                    )
                                nc.tensor.matmul(
                                    out=wtb[:], lhsT=vt, rhs=etb[:, h, :],
                                    start=first, stop=last,
                                )

                        for pi, pair in enumerate(kpairs):
                            sta = stp.tile([128, 2, 1024], F32, tag="sta",
                                           space="PSUM")
                            stb = stp.tile([128, 2, 256], F32, tag="stb",
                                           space="PSUM")
                            for h in (0, 1):
                                r, t = pair[h]
                                if (r, t) == (7, 9):
                                    continue
                                kt = kvf[:, r, 128 * t : 128 * (t + 1)]
                                for lo, hi in QA:
                                    nc.tensor.matmul(out=sta[:, h, lo:hi],
                                                     lhsT=kt, rhs=QT[:, lo:hi],
                                                     start=True, stop=True)
                                nc.tensor.matmul(out=stb[:, h, :], lhsT=kt,
                                                 rhs=QT[:, QB[0]:QB[1]],
                                                 start=True, stop=True)
                            eta = ep.tile([128, 2, 1024], BF16, tag="eta")
                            etb = ep.tile([128, 2, 256], BF16, tag="etb")
                            if pair[1] == (7, 9):
                                # pad pair: zero everything, exp only the 16
                                # real rows of half 0 = global nodes 9984..10000
                                nc.gpsimd.memset(eta[:], 0)
                                nc.gpsimd.memset(etb[:], 0)
                                nc.scalar.activation(eta[0:16, 0, :],
                                                     sta[0:16, 0, :], EXP,
                                                     scale=INV,
                                                     bias=negbeta[0:16])
                                nc.scalar.activation(etb[0:16, 0, :],
                                                     stb[0:16, 0, :], EXP,
                                                     scale=INV,
                                                     bias=negbeta[0:16])
                            else:
                                # per-half exp: the next pair's score matmuls
                                # for half 0 can start as soon as half 0 is
                                # read, keeping ACT off the PE critical path
                                nc.scalar.activation(eta[:, 0, :], sta[:, 0, :],
                                                     EXP, scale=INV,
                                                     bias=negbeta[:])
                                nc.scalar.activation(eta[:, 1, :], sta[:, 1, :],
                                                     EXP, scale=INV,
                                                     bias=negbeta[:])
                                nc.scalar.activation(etb[:], stb[:], EXP,
                                                     scale=INV, bias=negbeta[:])
                            # Z accumulation, two-level: bf16 sums of two
                            # pairs, f32 master accumulate every 2 pairs
                            if pi % 2 == 0:
                                held = (eta, etb)
                            else:
                                esum = ep.tile([128, 2, 1024], BF16, tag="esum")
                                ebsum = ep.tile([128, 2, 256], BF16, tag="ebsum")
                                nc.vector.tensor_tensor(out=esum[:],
                                                        in0=held[0][:],
                                                        in1=eta[:], op=ADD)
                                nc.vector.tensor_tensor(out=ebsum[:],
                                                        in0=held[1][:],
                                                        in1=etb[:], op=ADD)
                                if pi == 1:
                                    nc.vector.tensor_copy(out=Zacc[:],
                                                          in_=esum[:])
                                    nc.vector.tensor_copy(out=Zaccb[:],
                                                          in_=ebsum[:])
                                else:
                                    nc.vector.tensor_tensor(out=Zacc[:],
                                                            in0=Zacc[:],
                                                            in1=esum[:], op=ADD)
                                    nc.vector.tensor_tensor(out=Zaccb[:],
                                                            in0=Zaccb[:],
                                                            in1=ebsum[:],
                                                            op=ADD)
                            if pending is not None:
                                emit_pv(pending)
                            pending = (eta, etb, pi, pair)
                        emit_pv(pending)

                    # ---------- epilogue ----------
                    # Z broadcast: ones[128,128]^T @ Zacc puts sum_k in every
                    # output partition, keeping all DVE work 128 lanes wide
                    with tc.tile_pool(name="ez", bufs=1, space="PSUM") as ezp:
                        zbp = ezp.tile([128, ND], F32, tag="ez", space="PSUM")
                        for lo, hi in SL3:
                            for h in (0, 1):
                                zsrc = (Zacc[:, h, lo:hi] if hi <= 1024
                                        else Zaccb[:, h, 0:256])
                                nc.tensor.matmul(out=zbp[:, lo:hi],
                                                 lhsT=onesq[:], rhs=zsrc,
                                                 start=(h == 0), stop=(h == 1))
                        zb = sa.tile([128, ND], F32, tag="zb")
                        nc.vector.tensor_scalar_add(out=zb[:], in0=zbp[:],
                                                    scalar1=1e-30)
                        rz = sa.tile([128, ND], F32, tag="rz")
                        nc.vector.reciprocal_approx_fast(out=rz[:], in_=zb[:])
                        rb = sa.tile([128, ND], F32, tag="rb")
                        nc.vector.tensor_tensor(out=rb[:], in0=rz[:],
                                                in1=qm128[:], op=MULT)
                        wn = sa.tile([128, ND], F32, tag="wn")
                        nc.vector.tensor_tensor(out=wn[:, 0:1024], in0=wta[:],
                                                in1=rb[:, 0:1024], op=MULT)
                        nc.vector.tensor_tensor(out=wn[:, 1024:1280], in0=wtb[:],
                                                in1=rb[:, 1024:1280], op=MULT)
                        acc = sa.tile([128, 1], F32, tag="acc")
                        nc.vector.reduce_sum(acc[:], wn[:],
                                             axis=mybir.AxisListType.X)
                        nc.sync.dma_start(out=accb[:], in_=acc[:])
                        # AllGather + local sum beats AllReduce's latency floor
                        nc.gpsimd.collective_compute(
                            "AllGather", mybir.AluOpType.bypass, replica_groups=rg,
                            ins=[accb[:].opt()], outs=[accf[:].opt()],
                        )
                        accg = sa.tile([128, NCORES], F32, tag="accg")
                        for r in range(NCORES):
                            nc.sync.dma_start(out=accg[:, r : r + 1], in_=accf[r])
                        acc2 = sa.tile([128, 1], F32, tag="acc2")
                        nc.vector.reduce_sum(acc2[:], accg[:],
                                             axis=mybir.AxisListType.X)
                        aggc = sa.tile([128, 1], F32, tag="aggc")
                        nc.scalar.activation(aggc[:], acc2[:], IDENT,
                                             scale=1.0 / NREAL, bias=vgv[:])

                        # ---------- tiny MLP head (replicated) ----------
                        hd = ezp.tile([128, 512], F32, tag="hd", space="PSUM")
                        nc.tensor.matmul(out=hd[:, 0:1], lhsT=wo[:], rhs=aggc[:],
                                         start=True, stop=True)
                        state = sa.tile([128, 1], F32, tag="state")
                        nc.scalar.activation(state[:], hd[:, 0:1], IDENT, bias=bo[:])
                        hd2 = ezp.tile([128, 512], F32, tag="hd", space="PSUM")
                        nc.tensor.matmul(out=hd2[:64, 0:1], lhsT=wf1[:], rhs=state[:],
                                         start=True, stop=True)
                        x1 = sa.tile([64, 1], F32, tag="x1")
                        nc.scalar.activation(x1[:], hd2[:64, 0:1], RELU, bias=bf1[:])
                        hd3 = ezp.tile([128, 512], F32, tag="hd", space="PSUM")
                        nc.tensor.matmul(out=hd3[:32, 0:1], lhsT=wf2[:], rhs=x1[:],
                                         start=True, stop=True)
                        x2 = sa.tile([32, 1], F32, tag="x2")
                        nc.scalar.activation(x2[:], hd3[:32, 0:1], RELU, bias=bf2[:])
                        hd4 = ezp.tile([128, 512], F32, tag="hd", space="PSUM")
                        nc.tensor.matmul(out=hd4[:, 0:1], lhsT=wf3[:], rhs=x2[:],
                                         start=True, stop=True)
                        lg = sa.tile([128, 1], F32, tag="lg")
                        nc.scalar.activation(lg[:], hd4[:, 0:1], IDENT, bias=bf3[:])
                        hd5 = ezp.tile([128, 512], F32, tag="hd", space="PSUM")
                        nc.tensor.transpose(out=hd5[:1, 0:128], in_=lg[:],
                                            identity=ident[:])
                        er = sa.tile([1, 128], F32, tag="er")
                        zf = sa.tile([1, 1], F32, tag="zf")
                        nc.scalar.activation(er[:], hd5[:1, 0:128], EXP,
                                             accum_out=zf[:])
                        rzf = sa.tile([1, 1], F32, tag="rzf")
                        nc.vector.reciprocal(rzf[:], zf[:])
                        orow = sa.tile([1, 128], F32, tag="orow")
                        nc.vector.tensor_scalar(out=orow[:], in0=er[:],
                                                scalar1=rzf[:], scalar2=None,
                                                op0=MULT)
                        nc.sync.dma_start(out=out_t[:], in_=orow[:])

    nc.compile()
    return nc


def _get_nc():
    phase = int(os.environ.get("K_PHASE", "9"))
    key = ("nc", phase)
    if key not in _NC_CACHE:
        _NC_CACHE[key] = _build(phase)
    return _NC_CACHE[key]


def _prep_in_maps(inputs):
    f32 = np.float32
    x = np.asarray(inputs["node_features"], f32)
    g = np.asarray(inputs["global_info"], f32)
    ei = np.asarray(inputs["edge_index"])
    src = np.asarray(ei[0], np.int64)
    dst = np.asarray(ei[1], np.int64)

    xp = np.zeros((NP, D), f32)
    xp[:NREAL] = x
    xb = xp.astype(NP_BF16)
    x_tiled = np.ascontiguousarray(xb.reshape(SB, 128, D).transpose(1, 0, 2))

    qgv = (np.asarray(inputs["bQ"], f32)
           + (g @ np.asarray(inputs["WQg"], f32))[0]
           + np.asarray(inputs["bQg"], f32))
    vgv = (np.asarray(inputs["bV"], f32)
           + (g @ np.asarray(inputs["WVg"], f32))[0]
           + np.asarray(inputs["bVg"], f32))

    # host-side shift fit: c_q ~= Q_q . u + beta tracks the per-row max of
    # the (unshifted) scores so exp stays in fp32/bf16 range on device
    if _scipy_sparse is not None:
        adj = _scipy_sparse.csr_matrix(
            (np.ones(src.shape[0], f32), (dst, src)), shape=(NREAL, NREAL))
        segsum = lambda m: adj @ m
    else:
        order = np.argsort(dst, kind="stable")
        dsort = dst[order]
        ssort = src[order]
        starts = np.flatnonzero(np.r_[True, dsort[1:] != dsort[:-1]])
        uniq = dsort[starts]

        def segsum(m):
            out = np.zeros((NREAL, m.shape[1]), f32)
            out[uniq] = np.add.reduceat(m[ssort], starts, axis=0)
            return out

    W1r = np.asarray(inputs["W1_root"], f32)
    W1l = np.asarray(inputs["W1_rel"], f32)
    W2r = np.asarray(inputs["W2_root"], f32)
    W2l = np.asarray(inputs["W2_rel"], f32)
    h_h = np.maximum(x @ W1r + segsum(x) @ W1l + np.asarray(inputs["b1"], f32), 0)
    ne_h = h_h @ W2r + segsum(h_h) @ W2l + np.asarray(inputs["b2"], f32)
    Q_h = ne_h @ np.asarray(inputs["WQ"], f32) + qgv
    K_h = ne_h @ np.asarray(inputs["WK"], f32)          # device K has no bias
    rowmax_sub = ((Q_h @ K_h[::16].T) * INV).max(axis=1)
    Afit = np.hstack([Q_h, np.ones((NREAL, 1), f32)]).astype(np.float64)
    yfit = rowmax_sub.astype(np.float64)
    AtA = Afit.T @ Afit
    lam = 1e-4 * np.mean(np.diag(AtA)[:D])
    sol = np.linalg.solve(AtA + lam * np.eye(D + 1), Afit.T @ yfit)
    resid = rowmax_sub - (Afit @ sol).astype(f32)
    delta = float(resid.max()) - 8.0
    u = sol[:D].astype(f32)
    beta = float(sol[D]) + delta
    negv = (-(u / INV)).reshape(D, 1).astype(f32)
    negbeta = np.full((D, 1), -beta, f32)

    def bf(name):
        return np.ascontiguousarray(np.asarray(inputs[name], f32).astype(NP_BF16))

    shared = {
        "w1r": bf("W1_root"), "w1l": bf("W1_rel"),
        "w2r": bf("W2_root"), "w2l": bf("W2_rel"),
        "wq": bf("WQ"), "wk": bf("WK"), "wv": bf("WV"),
        "b1": np.asarray(inputs["b1"], f32).reshape(D, 1),
        "b2": np.asarray(inputs["b2"], f32).reshape(D, 1),
        "qgv": qgv.reshape(D, 1).copy(), "vgv": vgv.reshape(D, 1).copy(),
        "negv": negv, "negbeta": negbeta,
        "wo": np.asarray(inputs["Wo"], f32),
        "wf1": np.asarray(inputs["Wfc1"], f32),
        "wf2": np.asarray(inputs["Wfc2"], f32),
        "wf3": np.asarray(inputs["Wfc3"], f32),
        "bo": np.asarray(inputs["bo"], f32).reshape(D, 1),
        "bf1": np.asarray(inputs["bfc1"], f32).reshape(64, 1),
        "bf2": np.asarray(inputs["bfc2"], f32).reshape(32, 1),
        "bf3": np.asarray(inputs["bfc3"], f32).reshape(D, 1),
        "x_tiled": x_tiled,
    }

    core_of = dst // ND
    in_maps = []
    nodes = np.arange(NP)
    for c in range(NCORES):
        m = core_of == c
        A = np.zeros((NP, ND), f32)
        np.add.at(A, (src[m], dst[m] - ND * c), 1.0)
        Ac = np.ascontiguousarray(
            A.reshape(SB, 128, ND).transpose(1, 0, 2)
        ).astype(NP_FP8)
        xTmc = np.ascontiguousarray(xb[ND * c : ND * (c + 1)].T)
        qm = (nodes[ND * c : ND * (c + 1)] < NREAL).astype(f32)
        qm128 = np.ascontiguousarray(np.broadcast_to(qm.reshape(1, ND), (128, ND)))
        in_maps.append({**shared, "a_cnt": Ac, "xT_mine": xTmc,
                        "qmask128": qm128})
    return in_maps


def kernel(**inputs):
    nc = _get_nc()
    in_maps = _prep_in_maps(inputs)
    res = run_bass_kernel_spmd(nc, in_maps, core_ids=list(range(NCORES)))
    return np.asarray(res.results[0]["out"], np.float32)
